# revision 14
# baseline (speedup 1.0000x reference)
"""DFloat11 decompress + Linear (y = x @ W^T) on 8 Trainium2 NeuronCores.

Column-parallel sharding: each core decodes its 1376-row slice of the
compressed weight (sign_mantissa/exponent byte streams -> bf16) and
computes its output-feature slice of the GEMM. Outputs are concatenated
on the host (no collectives needed).

Device-side per core:
  - decode: ACT computes e*128; DVE assembles bf16 bit patterns
    (bits = sm + 128*e + 32640*(sm>=128), exact uint16 arithmetic)
    into an SBUF-resident [K=4096, N=1376] bf16 weight (bitcast).
  - x rows are loaded f32, converted to bf16 on DVE, staged to DRAM,
    then transposed K-major via dma_start_transpose (xbar).
  - PE: out[m,n] accumulated over 32 k-blocks in PSUM, x^T stationary,
    w^T moving, bf16 x bf16 -> f32.

Engine layout: ACT issues loads + does e*128 and PSUM->SBUF copies;
gpsimd (SWDGE) issues stores; sync issues only the xbar transposes
(they block their queue on long waits); DVE decodes + converts.
"""

import numpy as np

IN_F = 4096  # K
OUT_F = 11008  # N total
M = 4096  # 2*2048 tokens
NCORES = 8
NSH = OUT_F // NCORES  # 1376 out features per core

P = 128
KB = IN_F // P  # 32 k-blocks
KGRP = 2  # k-blocks per stream DMA
MCHUNK = 256
NMC = M // MCHUNK  # 16 m-chunks
MSUB = MCHUNK // P  # 2 m-subtiles per chunk
N_CHUNKS = [(0, 512), (512, 512), (1024, 352)]  # psum-bank sized n slices

_PROGRAM = None
LAST_RESULTS = None


def _build_program():
    import concourse.mybir as mybir
    import concourse.tile as tile
    from concourse import bacc

    dt = mybir.dt
    Alu = mybir.AluOpType

    nc = bacc.Bacc()
    x_d = nc.declare_dram_parameter("x", [M, IN_F], dt.float32, isOutput=False)
    smt_d = nc.declare_dram_parameter("smt", [IN_F, NSH], dt.uint16, isOutput=False)
    ext_d = nc.declare_dram_parameter("ext", [IN_F, NSH], dt.uint8, isOutput=False)
    y_d = nc.declare_dram_parameter("y", [M, NSH], dt.float32, isOutput=True)

    smt_g = smt_d.ap().rearrange("(g j p) c -> g p j c", j=KGRP, p=P)
    ext_g = ext_d.ap().rearrange("(g j p) c -> g p j c", j=KGRP, p=P)

    with tile.TileContext(nc) as tc:
        from contextlib import ExitStack

        with ExitStack() as ctx:
            wpool = ctx.enter_context(tc.tile_pool(name="w", bufs=1))
            dec = ctx.enter_context(tc.tile_pool(name="dec", bufs=2))
            xtp = ctx.enter_context(tc.tile_pool(name="xt", bufs=3))
            ypool = ctx.enter_context(tc.tile_pool(name="yp", bufs=2))
            psum = ctx.enter_context(tc.tile_pool(name="ps", bufs=2, space="PSUM"))
            dram = ctx.enter_context(tc.tile_pool(name="dr", bufs=1, space="DRAM"))
            xfp = ctx.enter_context(tc.tile_pool(name="xf", bufs=2))
            xbfp = ctx.enter_context(tc.tile_pool(name="xbf", bufs=2))

            # ---- weight decode into one resident bf16 tensor [128, KB, NSH]
            w_big = wpool.tile([P, KB, NSH], dt.bfloat16, tag="w", name="w_big")
            w_u16 = w_big.bitcast(dt.uint16)
            for g in range(KB // KGRP):
                sm = dec.tile([P, KGRP, NSH], dt.uint16, tag="sm", name="sm")
                ex = dec.tile([P, KGRP, NSH], dt.uint8, tag="ex", name="ex")
                nc.gpsimd.dma_start(sm[:], smt_g[g])
                nc.scalar.dma_start(ex[:], ext_g[g])
                for j in range(KGRP):
                    kb = g * KGRP + j
                    e128 = dec.tile([P, NSH], dt.int16, tag="e128", name="e128")
                    nc.scalar.mul(e128[:], ex[:, j, :], 128.0)
                    sb = dec.tile([P, NSH], dt.uint16, tag="sb", name="sb")
                    # 32640 * (sm >= 128)
                    nc.vector.tensor_scalar(sb[:], sm[:, j, :], 127.5, 32640.0, op0=Alu.is_ge, op1=Alu.mult)
                    # w1 = sm + 128*e (in-place over e128; values fit int16)
                    nc.vector.tensor_tensor(out=e128[:], in0=sm[:, j, :], in1=e128[:], op=Alu.add)
                    # bits = sm + 128*e + 32640*s  (== bf16 bit pattern)
                    nc.vector.tensor_tensor(out=w_u16[:, kb, :], in0=e128[:], in1=sb[:], op=Alu.add)

            # ---- per m-chunk: convert x rows to bf16, stage, transpose, GEMM
            for mc in range(NMC):
                xb = dram.tile([MCHUNK, IN_F], dt.bfloat16, tag=f"xb{mc}", name=f"xb{mc}")
                for ms in range(MSUB):
                    r0 = mc * MCHUNK + ms * P
                    xbf = xbfp.tile([P, IN_F], dt.bfloat16, tag="xbf", name="xbf")
                    for st in range(2):
                        c0 = st * (IN_F // 2)
                        xf = xfp.tile([P, IN_F // 2], dt.float32, tag="xf", name="xf")
                        nc.gpsimd.dma_start(xf[:], x_d[r0:r0 + P, c0:c0 + IN_F // 2])
                        nc.vector.tensor_copy(xbf[:, c0:c0 + IN_F // 2], xf[:])
                    nc.gpsimd.dma_start(xb[ms * P:(ms + 1) * P, :], xbf[:])

                xt = xtp.tile([P, KB, MCHUNK], dt.bfloat16, tag="xt", name="xt")
                xpose_eng = nc.sync if (mc % 2 == 0) else nc.scalar
                xpose_eng.dma_start_transpose(xt[:], xb[:])
                for ms in range(MSUB):
                    pts = []
                    for ni, (n0, nw) in enumerate(N_CHUNKS):
                        pts.append(psum.tile([P, nw], dt.float32, tag=f"ps{ni}", name=f"ps{ni}"))
                    for kb in range(KB):
                        lhsT = xt[:, kb, ms * P:(ms + 1) * P]
                        for ni, (n0, nw) in enumerate(N_CHUNKS):
                            nc.tensor.matmul(
                                pts[ni][:],
                                lhsT,
                                w_big[:, kb, n0:n0 + nw],
                                start=(kb == 0),
                                stop=(kb == KB - 1),
                            )
                    ysb = ypool.tile([P, NSH], dt.float32, tag="y", name="ysb")
                    for ni, (n0, nw) in enumerate(N_CHUNKS):
                        nc.vector.tensor_copy(ysb[:, n0:n0 + nw], pts[ni][:])
                    m0 = mc * MCHUNK + ms * P
                    nc.gpsimd.dma_start(y_d[m0:m0 + P, :], ysb[:])

    nc.finalize()
    return nc


def _get_program():
    global _PROGRAM
    if _PROGRAM is None:
        _PROGRAM = _build_program()
    return _PROGRAM


def _host_prep(x, sign_mantissa, exponent):
    x2d = np.ascontiguousarray(np.asarray(x, dtype=np.float32).reshape(M, IN_F))
    sm = np.asarray(sign_mantissa).astype(np.uint16).reshape(OUT_F, IN_F)
    ex = np.asarray(exponent).astype(np.uint8).reshape(OUT_F, IN_F)
    in_maps = []
    for c in range(NCORES):
        rows = slice(c * NSH, (c + 1) * NSH)
        smt = np.ascontiguousarray(sm[rows, :].T)  # [K, NSH] u16
        ext = np.ascontiguousarray(ex[rows, :].T)  # [K, NSH] u8
        in_maps.append({"x": x2d, "smt": smt, "ext": ext})
    return in_maps


def _run(in_maps, trace=False):
    from concourse.bass_utils import run_bass_kernel_spmd

    nc = _get_program()
    res = run_bass_kernel_spmd(nc, in_maps, list(range(NCORES)), trace=trace)
    return res


def kernel(x, sign_mantissa, exponent):
    global LAST_RESULTS
    import os

    in_maps = _host_prep(x, sign_mantissa, exponent)
    trace = bool(os.environ.get("KERNEL_TRACE"))
    res = _run(in_maps, trace=trace)
    LAST_RESULTS = res
    parts = [res.results[c]["y"] for c in range(NCORES)]
    y = np.concatenate(parts, axis=1).reshape(2, 2048, OUT_F)
    return np.ascontiguousarray(y.astype(np.float32))


# revision 16
# speedup vs baseline: 1.0314x; 1.0314x over previous
"""DFloat11 decompress + Linear (y = x @ W^T) on 8 Trainium2 NeuronCores.

Column-parallel sharding: each core decodes its 1376-row slice of the
compressed weight (sign_mantissa/exponent byte streams -> bf16) and
computes its output-feature slice of the GEMM. Outputs are concatenated
on the host (no collectives needed).

Device-side per core:
  - decode: ACT computes e*128; DVE assembles bf16 bit patterns
    (bits = sm + 128*e + 32640*(sm>=128), exact uint16 arithmetic)
    into an SBUF-resident [K=4096, N=1376] bf16 weight (bitcast).
  - x rows are loaded f32, converted to bf16 on DVE, staged to DRAM,
    then transposed K-major via dma_start_transpose (xbar).
  - PE: out[m,n] accumulated over 32 k-blocks in PSUM, x^T stationary,
    w^T moving, bf16 x bf16 -> f32.

Engine layout: ACT issues loads + does e*128 and PSUM->SBUF copies;
gpsimd (SWDGE) issues stores; sync issues only the xbar transposes
(they block their queue on long waits); DVE decodes + converts.
"""

import numpy as np

IN_F = 4096  # K
OUT_F = 11008  # N total
M = 4096  # 2*2048 tokens
NCORES = 8
NSH = OUT_F // NCORES  # 1376 out features per core

P = 128
KB = IN_F // P  # 32 k-blocks
KGRP = 2  # k-blocks per stream DMA
MCHUNK = 256
NMC = M // MCHUNK  # 16 m-chunks
MSUB = MCHUNK // P  # 2 m-subtiles per chunk
N_CHUNKS = [(0, 512), (512, 512), (1024, 352)]  # psum-bank sized n slices

_PROGRAM = None
LAST_RESULTS = None


def _build_program():
    import concourse.mybir as mybir
    import concourse.tile as tile
    from concourse import bacc

    dt = mybir.dt
    Alu = mybir.AluOpType

    nc = bacc.Bacc()
    x_d = nc.declare_dram_parameter("x", [M, IN_F], dt.float32, isOutput=False)
    smt_d = nc.declare_dram_parameter("smt", [IN_F, NSH], dt.uint16, isOutput=False)
    ext_d = nc.declare_dram_parameter("ext", [IN_F, NSH], dt.uint8, isOutput=False)
    y_d = nc.declare_dram_parameter("y", [M, NSH], dt.float32, isOutput=True)

    smt_g = smt_d.ap().rearrange("(g j p) c -> g p j c", j=KGRP, p=P)
    ext_g = ext_d.ap().rearrange("(g j p) c -> g p j c", j=KGRP, p=P)

    with tile.TileContext(nc) as tc:
        from contextlib import ExitStack

        with ExitStack() as ctx:
            wpool = ctx.enter_context(tc.tile_pool(name="w", bufs=1))
            dec = ctx.enter_context(tc.tile_pool(name="dec", bufs=2))
            xtp = ctx.enter_context(tc.tile_pool(name="xt", bufs=2))
            ypool = ctx.enter_context(tc.tile_pool(name="yp", bufs=2))
            psum = ctx.enter_context(tc.tile_pool(name="ps", bufs=2, space="PSUM"))
            dram = ctx.enter_context(tc.tile_pool(name="dr", bufs=1, space="DRAM"))
            xfp = ctx.enter_context(tc.tile_pool(name="xf", bufs=2))
            xbfp = ctx.enter_context(tc.tile_pool(name="xbf", bufs=2))

            # ---- weight decode into one resident bf16 tensor [128, KB, NSH]
            w_big = wpool.tile([P, KB, NSH], dt.bfloat16, tag="w", name="w_big")
            w_u16 = w_big.bitcast(dt.uint16)
            for g in range(KB // KGRP):
                sm = dec.tile([P, KGRP, NSH], dt.uint16, tag="sm", name="sm")
                ex = dec.tile([P, KGRP, NSH], dt.uint8, tag="ex", name="ex")
                nc.gpsimd.dma_start(sm[:], smt_g[g])
                nc.gpsimd.dma_start(ex[:], ext_g[g])
                for j in range(KGRP):
                    kb = g * KGRP + j
                    e128 = dec.tile([P, NSH], dt.int16, tag="e128", name="e128")
                    nc.scalar.mul(e128[:], ex[:, j, :], 128.0)
                    sb = dec.tile([P, NSH], dt.uint16, tag="sb", name="sb")
                    # 32640 * (sm >= 128)
                    nc.vector.tensor_scalar(sb[:], sm[:, j, :], 127.5, 32640.0, op0=Alu.is_ge, op1=Alu.mult)
                    # w1 = sm + 128*e (in-place over e128; values fit int16)
                    nc.vector.tensor_tensor(out=e128[:], in0=sm[:, j, :], in1=e128[:], op=Alu.add)
                    # bits = sm + 128*e + 32640*s  (== bf16 bit pattern)
                    nc.vector.tensor_tensor(out=w_u16[:, kb, :], in0=e128[:], in1=sb[:], op=Alu.add)

            # ---- per m-chunk: convert x rows to bf16, stage, transpose, GEMM
            for mc in range(NMC):
                xb = dram.tile([MCHUNK, IN_F], dt.bfloat16, tag=f"xb{mc}", name=f"xb{mc}")
                for ms in range(MSUB):
                    r0 = mc * MCHUNK + ms * P
                    xf = xfp.tile([P, IN_F], dt.float32, tag="xf", name="xf")
                    nc.gpsimd.dma_start(xf[:], x_d[r0:r0 + P, :])
                    xbf = xbfp.tile([P, IN_F], dt.bfloat16, tag="xbf", name="xbf")
                    nc.vector.tensor_copy(xbf[:], xf[:])
                    nc.gpsimd.dma_start(xb[ms * P:(ms + 1) * P, :], xbf[:])

                xt = xtp.tile([P, KB, MCHUNK], dt.bfloat16, tag="xt", name="xt")
                nc.sync.dma_start_transpose(xt[:], xb[:])
                for ms in range(MSUB):
                    pts = []
                    for ni, (n0, nw) in enumerate(N_CHUNKS):
                        pts.append(psum.tile([P, nw], dt.float32, tag=f"ps{ni}", name=f"ps{ni}"))
                    for kb in range(KB):
                        lhsT = xt[:, kb, ms * P:(ms + 1) * P]
                        for ni, (n0, nw) in enumerate(N_CHUNKS):
                            nc.tensor.matmul(
                                pts[ni][:],
                                lhsT,
                                w_big[:, kb, n0:n0 + nw],
                                start=(kb == 0),
                                stop=(kb == KB - 1),
                            )
                    ysb = ypool.tile([P, NSH], dt.float32, tag="y", name="ysb")
                    for ni, (n0, nw) in enumerate(N_CHUNKS):
                        nc.scalar.copy(ysb[:, n0:n0 + nw], pts[ni][:])
                    m0 = mc * MCHUNK + ms * P
                    nc.gpsimd.dma_start(y_d[m0:m0 + P, :], ysb[:])

    nc.finalize()
    return nc


def _get_program():
    global _PROGRAM
    if _PROGRAM is None:
        _PROGRAM = _build_program()
    return _PROGRAM


def _host_prep(x, sign_mantissa, exponent):
    x2d = np.ascontiguousarray(np.asarray(x, dtype=np.float32).reshape(M, IN_F))
    sm = np.asarray(sign_mantissa).astype(np.uint16).reshape(OUT_F, IN_F)
    ex = np.asarray(exponent).astype(np.uint8).reshape(OUT_F, IN_F)
    in_maps = []
    for c in range(NCORES):
        rows = slice(c * NSH, (c + 1) * NSH)
        smt = np.ascontiguousarray(sm[rows, :].T)  # [K, NSH] u16
        ext = np.ascontiguousarray(ex[rows, :].T)  # [K, NSH] u8
        in_maps.append({"x": x2d, "smt": smt, "ext": ext})
    return in_maps


def _run(in_maps, trace=False):
    from concourse.bass_utils import run_bass_kernel_spmd

    nc = _get_program()
    res = run_bass_kernel_spmd(nc, in_maps, list(range(NCORES)), trace=trace)
    return res


def kernel(x, sign_mantissa, exponent):
    global LAST_RESULTS
    import os

    in_maps = _host_prep(x, sign_mantissa, exponent)
    trace = bool(os.environ.get("KERNEL_TRACE"))
    res = _run(in_maps, trace=trace)
    LAST_RESULTS = res
    parts = [res.results[c]["y"] for c in range(NCORES)]
    y = np.concatenate(parts, axis=1).reshape(2, 2048, OUT_F)
    return np.ascontiguousarray(y.astype(np.float32))
